# revision 1
# baseline (speedup 1.0000x reference)
"""Bass/Trainium2 kernel for nn_LogReg_8151847928094.

out[b] = sum_s w[text[s, b]] + bias   (bag-of-words logistic regression)

Strategy (8 NeuronCores, batch-sharded 2048 -> 8 x 256 columns):
  - Split token t = hi*128 + lo (hi < 782). A token can be served by one
    of two Q7 core groups: gA = lo//16 (lane lo%16, table half A) or
    gB = lo%8 (lane lo//8, table half B). The host greedily balances
    per-(column, group) run lengths across the two choices.
  - SBUF table per partition p = 16g + l (two lane-expanded halves,
    zero everywhere except the partition's own lane slot, plus a zero
    pad column h = 782):
        A: data[p, l'*783 + h]         = w[h*128 + 16g + l]  iff l' == l
        B: data[p, 12528 + l'*783 + h] = w[h*128 + 8*l + g]  iff l' == l
  - Host packs each group's tokens into per-batch-column runs of R
    slots; index value encodes (half, lane, hi); pad slots point at a
    zero entry. One gpsimd.ap_gather per core: out[p, i] =
    data[p, idx_g(i)] puts each token's w value on exactly its lane,
    zeros on the other lanes of the group.
  - DVE segmented reduce over runs -> [128, 256]; PE ones-matmul
    reduces partitions -> [1, 256]; add bias; DMA out.
"""

import sys

sys.path.insert(0, "/opt/trn_rl_repo")

import numpy as np

import concourse.bass as bass
import concourse.bacc as bacc
import concourse.mybir as mybir
import concourse.tile as tile
from concourse.bass_utils import run_bass_kernel_spmd

S = 200
B = 2048
V = 100000
NCORES = 8
BS = B // NCORES  # 256 batch columns per core
P = 128
HI = (V + P - 1) // P  # 782
HIP = HI + 1  # 783, column HI of each lane slot is zero (pad target)
HTBL = 16 * HIP  # 12528 entries per table half
TBL = 2 * HTBL  # 25056 table entries per partition (~100 KB)

_prog_cache = {}


def _build_program(R, loop_T=None):
    NIDX = BS * R  # stream length per group
    NW = NIDX // 16
    nc = bacc.Bacc("TRN2", target_bir_lowering=False, debug=False)
    idxs_d = nc.declare_dram_parameter("idxs", [P, NW], mybir.dt.int16, isOutput=False)
    wtl_d = nc.declare_dram_parameter(
        "wtl", [16, 2, 8, HIP], mybir.dt.float32, isOutput=False
    )
    bias_d = nc.declare_dram_parameter("bias", [1, BS], mybir.dt.float32, isOutput=False)
    out_d = nc.declare_dram_parameter("out", [1, BS], mybir.dt.float32, isOutput=True)

    with tile.TileContext(nc) as tc:
        with (
            tc.tile_pool(name="sbuf", bufs=1) as pool,
            tc.tile_pool(name="psum", bufs=1, space="PSUM") as psum_pool,
        ):
            table_t = pool.tile([P, TBL], mybir.dt.float32)
            idxs_t = pool.tile([P, NW], mybir.dt.int16)
            gath_t = pool.tile([P, NIDX], mybir.dt.float32)
            red_t = pool.tile([P, BS], mybir.dt.float32)
            ones_t = pool.tile([P, 1], mybir.dt.float32)
            bias_t = pool.tile([1, BS], mybir.dt.float32)
            res_t = pool.tile([1, BS], mybir.dt.float32)
            psum_t = psum_pool.tile([1, BS], mybir.dt.float32)

            def body():
                nc.gpsimd.memset(table_t[:], 0)
                nc.gpsimd.memset(ones_t[:], 1.0)
                nc.sync.dma_start(out=idxs_t[:], in_=idxs_d[:])
                nc.sync.dma_start(out=bias_t[:], in_=bias_d[:])
                # Non-zero lane stripes: partitions l, l+16, ... get their
                # own 783-entry slice in each half.
                lanes = table_t[:].rearrange("(a l) f -> l a f", l=16)
                for l in range(16):
                    nc.sync.dma_start(
                        out=lanes[l][:, l * HIP : (l + 1) * HIP],
                        in_=wtl_d[l, 0],
                    )
                    nc.sync.dma_start(
                        out=lanes[l][:, HTBL + l * HIP : HTBL + (l + 1) * HIP],
                        in_=wtl_d[l, 1],
                    )
                nc.gpsimd.ap_gather(
                    gath_t[:],
                    table_t[:],
                    idxs_t[:],
                    channels=P,
                    num_elems=TBL,
                    d=1,
                    num_idxs=NIDX,
                )
                nc.vector.tensor_reduce(
                    out=red_t[:],
                    in_=gath_t[:].rearrange("p (b r) -> p b r", r=R),
                    axis=mybir.AxisListType.X,
                    op=mybir.AluOpType.add,
                )
                nc.tensor.matmul(
                    psum_t[:], lhsT=ones_t[:], rhs=red_t[:], start=True, stop=True
                )
                nc.vector.tensor_tensor(
                    out=res_t[:], in0=psum_t[:], in1=bias_t[:], op=mybir.AluOpType.add
                )
                nc.sync.dma_start(out=out_d[:], in_=res_t[:])

            if loop_T is None:
                body()
            else:
                with tc.For_i(0, loop_T, 1) as _i:
                    body()
    nc.compile()
    return nc


def _balance_core(tokens):
    """tokens [S, BS] int -> (g_fin, idxv, b_of) flat, b-major order."""
    t = tokens.astype(np.int64).ravel(order="F")
    b_of = np.repeat(np.arange(BS, dtype=np.int64), S)
    lo = t % P
    hi = t // P
    gA = (lo // 16).astype(np.int64)
    idxA = (lo % 16) * HIP + hi
    gB = (lo % 8).astype(np.int64)
    idxB = HTBL + (lo // 8) * HIP + hi
    sel = np.empty(t.size, np.bool_)  # True -> choice B
    loads = np.zeros(8, np.int32)
    for col in range(BS):
        base = col * S
        loads[:] = 0
        ga = gA[base : base + S]
        gb = gB[base : base + S]
        sl = sel[base : base + S]
        for i in range(S):
            a = ga[i]
            bb = gb[i]
            if loads[a] <= loads[bb]:
                sl[i] = False
                loads[a] += 1
            else:
                sl[i] = True
                loads[bb] += 1
        # Iterative refinement: shed tokens from the fullest group to
        # their alternative whenever that strictly lowers the max.
        for _ in range(48):
            top = int(loads.argmax())
            moved = False
            for i in range(S):
                cur = gb[i] if sl[i] else ga[i]
                alt = ga[i] if sl[i] else gb[i]
                if cur == top and alt != top and loads[alt] + 1 < loads[top]:
                    sl[i] = not sl[i]
                    loads[top] -= 1
                    loads[alt] += 1
                    moved = True
                    break
            if not moved:
                break
    g_fin = np.where(sel, gB, gA)
    idxv = np.where(sel, idxB, idxA).astype(np.int16)
    return g_fin, idxv, b_of


def _pack_core(g_fin, idxv, b_of, R):
    key = g_fin * BS + b_of
    order = np.argsort(key, kind="stable")
    ks = key[order]
    starts = np.r_[0, np.flatnonzero(np.diff(ks)) + 1]
    run_ids = np.cumsum(np.r_[0, np.diff(ks) != 0])
    rank = np.arange(ks.size) - starts[run_ids]
    NIDX = BS * R
    streams = np.full((8, NIDX), HI, dtype=np.int16)  # pad -> zero column
    streams[ks // BS, (ks % BS) * R + rank] = idxv[order]
    idxs = np.empty((P, NIDX // 16), np.int16)
    for g in range(8):
        idxs[16 * g : 16 * g + 16, :] = streams[g].reshape(NIDX // 16, 16).T
    return idxs


def kernel(text, w, b):
    text = np.asarray(text)
    w = np.asarray(w, dtype=np.float32).reshape(-1)
    b = np.asarray(b, dtype=np.float32).reshape(-1)

    assigns = []
    R = 0
    for c in range(NCORES):
        g_fin, idxv, b_of = _balance_core(text[:, c * BS : (c + 1) * BS])
        assigns.append((g_fin, idxv, b_of))
        cnt = np.bincount(g_fin * BS + b_of, minlength=8 * BS)
        R = max(R, int(cnt.max()))

    nc = _prog_cache.get(R)
    if nc is None:
        nc = _build_program(R)
        _prog_cache[R] = nc

    # Table lane slices. wr[h, p] = w[h*128 + p] (zeros pad h = HI row).
    w_pad = np.zeros(HIP * P, np.float32)
    w_pad[:V] = w
    wr = w_pad.reshape(HIP, P)
    wtlA = np.ascontiguousarray(wr.T.reshape(8, 16, HIP).transpose(1, 0, 2))
    wtlB = np.ascontiguousarray(wr.T.reshape(16, 8, HIP))
    wtl = np.stack([wtlA, wtlB], axis=1)  # [16, 2, 8, HIP]
    bias_row = np.full((1, BS), b[0], np.float32)

    in_maps = []
    for c in range(NCORES):
        idxs = _pack_core(*assigns[c], R)
        in_maps.append({"idxs": idxs, "wtl": wtl, "bias": bias_row})

    res = run_bass_kernel_spmd(nc, in_maps, list(range(NCORES))).results
    out = np.concatenate([res[c]["out"][0] for c in range(NCORES)])
    return out.astype(np.float32)


if __name__ == "__main__":
    rng = np.random.default_rng(0)
    text = rng.integers(0, V, (S, B)).astype(np.int64)
    w = rng.standard_normal((1, V)).astype(np.float32) * 0.01
    b = np.zeros((1,), np.float32)
    out = kernel(text, w, b)
    exp = w[0][text].sum(axis=0) + b[0]
    err = np.abs(out - exp).max() / (np.abs(exp).max() + 1e-9)
    print("rel err:", err)

